# revision 27
# baseline (speedup 1.0000x reference)
"""DescriptorLoss kernel for Trainium2 (8 NeuronCores, SPMD data-parallel).

Math:
    d[b,ij,kl] = sum_c desc0[b,c,ij] * desc1[b,c,kl]
    loss = mean(where(mask, 250*relu(1 - d), relu(d - 0.2)))

Per core (shard = (batch, i-slab) -> 1024 ij rows x 4096 kl cols), the PE
computes d' = 5*d via fp8 matmuls into PSUM fp32 in 32 chunks of
[128 x 1024] (psum pool depth 4).  In d' units the hinges sit at 1 and 5:
    5*loss_elem = relu(d'-1)        if m == 0
                  250*relu(5-d')    if m == 1

23 chunks go to the DVE (one fused custom op per chunk, Src1 = t =
(m ? 8192 : 1) fp8 e5m2):
    body = relu(d' - t) + relu((t - d' - 8187) * 250),  accum = sum
  m=0: relu(d'-1); m=1: 250*relu(5-d').  One PSUM read per element.

9 chunks go to ACT: the PE injects the mask into PSUM
(psum += (-8192*I).T @ m) giving dM = d' - 8192*m; ACT runs two relu
passes with the 250 weight folded into the free affine:
    acc1 = sum relu(dM - 1)              = sum_{m=0} relu(d'-1)
    acc2 = sum relu(-250*dM - 250*8187)  = 250 * sum_{m=1} relu(5-d')

Scheduling notes (learned from traces):
  - Chunks are processed h-minor (all h=0 column blocks first), so the
    single leading DMA (aw ++ b[:, :1024] ++ first two DVE masks) feeds
    the first 8 matmuls and the hinge pipeline starts ~1us after data
    lands.  All transfers ride ONE sync-HWDGE ring in exact consumption
    order: packet-level round-robin between rings/queues would otherwise
    delay early transfers by later ones.
  - Exactly 8 input DMAs: the Tile scheduler has 8 DMA-completion
    semaphore lanes; more input DMAs alias lanes and create false
    multi-microsecond waits on unrelated transfers.
  - The scalar engine issues no DMAs (descriptor generation costs
    ~0.7us/DMA on the issuing engine) so ACT is free for hinge passes.
  - No PE warmup: cold matmul rate still outpaces the DVE hinge stream,
    and sustained real matmuls open the HAM clock gate on their own.
"""

import numpy as np
import ml_dtypes
from operator import add

import concourse.bacc as bacc
import concourse.mybir as mybir
import concourse.tile as tile
import concourse.dve_ops as dve_ops_mod
from concourse.dve_spec import Spec, Src0, Src1, C0, C1, relu, lower
from concourse.dve_uop import DveOpSpec
from concourse.bass_utils import run_bass_kernel_spmd

B, D, H, W = 2, 128, 64, 64
N_CORES = 8
IJ = H * W                # 4096
ROWS_PER_CORE = IJ // 4   # 1024
G = ROWS_PER_CORE // 128  # 8 row groups of 128
CH = 1024                 # chunk columns
KT = IJ // CH             # 4 chunks per row group
N_CHUNKS = G * KT         # 32
MOFF = 8192.0             # mask offset (exact in fp8 e5m2)
LAM = 250.0

# processing order: h-minor (all h=0 chunks first), g-major within h
ORDER = tuple(g * KT + h for h in range(KT) for g in range(G))
ACT_POS = (2, 5, 9, 12, 16, 19, 23, 26, 30)      # positions on ACT (9)
DVE_POS = tuple(p for p in range(N_CHUNKS) if p not in ACT_POS)
# DVE mask windows, in processing order: 2 chunks ride in the lead DMA
DVE_GROUPS = (2, 3, 6, 6, 6)
MACT_SPLIT = 2                                   # ACT chunks in mi0

_cached = {}

_OP_NAME = "HINGE_PAIR_MASKED_ANT"


def _hinge_ref(in0, in1, s0, s1, imm2):
    x = in0.astype(np.float32)
    t = in1.astype(np.float32)
    out = np.maximum(x - t, 0) + np.maximum((t - x - s0) * s1, 0)
    return out, out.reshape(out.shape[0], -1).sum(axis=-1, keepdims=True).astype(
        np.float32
    )


def _register_dve_op():
    """Register the fused two-hinge op in dve_ops.OPS (documented extension
    point; the uop table is emitted per-NEFF at compile time)."""
    for op in dve_ops_mod.OPS:
        if op.name == _OP_NAME:
            return op
    spec = Spec(
        body=relu(Src0 - Src1) + relu((Src1 - Src0 - C0) * C1),
        accum=add,
        reference=_hinge_ref,
    )
    opcode = dve_ops_mod._CUSTOM_DVE_ROW_BASE + len(dve_ops_mod.OPS)
    shas = {}
    for ver in ("v3", "v4"):
        shas[ver] = DveOpSpec(
            name=_OP_NAME, opcode=opcode, uops=lower(spec, ver=ver), rd1_en=True
        ).sha(ver)
    op = dve_ops_mod.DveOp(_OP_NAME, spec, subdim=False, uops_sha=shas)
    dve_ops_mod.OPS.append(op)
    dve_ops_mod._SUB_OPCODE_FOR_NAME[_OP_NAME] = opcode
    dve_ops_mod.CUSTOM_DVE_SPECS[_OP_NAME] = spec
    return op


_HINGE_OP = _register_dve_op()


def _build_program():
    nc = bacc.Bacc("TRN2")
    f32 = mybir.dt.float32
    bf16 = mybir.dt.bfloat16
    f8 = mybir.dt.float8e5
    f8e4 = mybir.dt.float8e4
    Act = mybir.ActivationFunctionType
    n_act = len(ACT_POS)

    # lead_a = aw [128,1024] ++ b[:, :1024]; lead_m = first 2 DVE masks
    lead_a = nc.declare_dram_parameter("lead_a", [D, 2 * CH], f8e4, isOutput=False)
    lead_m = nc.declare_dram_parameter("lead_m", [D, 2 * CH], f8, isOutput=False)
    b1 = nc.declare_dram_parameter("b1", [D, 3 * CH], f8e4, isOutput=False)
    mi0 = nc.declare_dram_parameter(
        "mi0", [D, D + MACT_SPLIT * CH], f8, isOutput=False)
    mi1 = nc.declare_dram_parameter(
        "mi1", [D, (n_act - MACT_SPLIT) * CH], f8, isOutput=False)
    mvs = [
        nc.declare_dram_parameter(f"mv{i}", [128, n * CH], f8, isOutput=False)
        for i, n in enumerate(DVE_GROUPS[1:])
    ]
    accs_out = nc.declare_dram_parameter("accs", [128, 64], f32, isOutput=True)

    # DVE position -> (window, col offset); window -1 = lead DMA
    dve_loc = {}
    k = 0
    for gq, n in enumerate(DVE_GROUPS):
        for idx in range(n):
            dve_loc[DVE_POS[k]] = (gq - 1, idx * CH)
            k += 1
    act_idx = {p: j for j, p in enumerate(ACT_POS)}

    with tile.TileContext(nc) as tc:
        with (
            tc.tile_pool(name="desc", bufs=1) as desc_pool,
            tc.tile_pool(name="mask", bufs=6) as mask_pool,
            tc.tile_pool(name="scr", bufs=4) as scr_pool,
            tc.tile_pool(name="accs", bufs=1) as acc_pool,
            tc.tile_pool(name="psd", bufs=4, space="PSUM") as psum_pool,
        ):
            lead_a_t = desc_pool.tile([D, 2 * CH], f8e4, tag="leada")
            lead_m_t = desc_pool.tile([D, 2 * CH], f8, tag="leadm")
            b1_t = desc_pool.tile([D, 3 * CH], f8e4, tag="b1")
            mi0_t = desc_pool.tile([D, D + MACT_SPLIT * CH], f8, tag="mi0")
            mi1_t = desc_pool.tile([D, (n_act - MACT_SPLIT) * CH], f8, tag="mi1")
            mgrp = [
                mask_pool.tile([128, n * CH], f8, tag="m", name=f"mg{i}")
                for i, n in enumerate(DVE_GROUPS[1:])
            ]
            warm = desc_pool.tile([128, 8], bf16, tag="warm")
            warm2 = desc_pool.tile([128, 8], bf16, tag="warm2")
            bias_a = desc_pool.tile([128, 1], f32, tag="ba")
            bias_b = desc_pool.tile([128, 1], f32, tag="bb")
            junk = desc_pool.tile([128, 512], f8e4, tag="junk")
            accD_t = acc_pool.tile([128, 32], f32, tag="accD")
            accA_t = acc_pool.tile([128, 32], f32, tag="accA")

            # single sync ring, exact consumption order
            nc.sync.dma_start(lead_a_t[:], lead_a[:])
            nc.sync.dma_start(lead_m_t[:], lead_m[:])
            nc.sync.dma_start(mi0_t[:], mi0[:])
            nc.sync.dma_start(mgrp[0][:], mvs[0][:])
            nc.sync.dma_start(mgrp[1][:], mvs[1][:])
            nc.sync.dma_start(b1_t[:], b1[:])
            nc.sync.dma_start(mi1_t[:], mi1[:])
            nc.sync.dma_start(mgrp[2][:], mvs[2][:])
            nc.sync.dma_start(mgrp[3][:], mvs[3][:])

            nc.vector.memset(junk[:], 0.0)
            nc.vector.memset(warm[:], 0.0)
            nc.vector.memset(bias_a[:], -1.0)
            nc.vector.memset(bias_b[:], -(LAM * (MOFF - 5.0)))
            nc.vector.memset(accD_t[:, 23:], 0.0)
            nc.vector.memset(accA_t[:, 2 * n_act:], 0.0)
            # prime the ACT relu table (~2.7us one-time) under the input DMAs
            nc.scalar.activation(warm2[:], warm[:], Act.Relu, bias=bias_a[:], scale=1.0)

            # HAM warmup: dense junk matmuls under the input-DMA shadow so
            # the PE clock gate is open (K=8/8) when the real matmuls begin.
            # They use the first psum pool buffer; the pos-3 chunk reuses it
            # only after they retire (~well before its data arrives).
            junk_ps = psum_pool.tile([128, CH], f32, tag="d")
            with tc.high_priority():
                for w in range(8):
                    nc.tensor.matmul(
                        junk_ps[:, (w % 2) * 512:(w % 2) * 512 + 512],
                        junk[:, 0:128], junk[:],
                        start=True, stop=True,
                    )

            aw_all = lead_a_t[:, 0:CH]
            b0 = lead_a_t[:, CH:2 * CH]

            n_dve = 0
            for pos in range(N_CHUNKS):
                cid = ORDER[pos]
                on_act = pos in ACT_POS
                g, h = divmod(cid, KT)
                psum_d = psum_pool.tile([128, CH], f32, tag="d")
                for s in range(2):
                    if h == 0:
                        rhs = b0[:, s * 512:(s + 1) * 512]
                    else:
                        c0 = (h - 1) * CH + s * 512
                        rhs = b1_t[:, c0:c0 + 512]
                    nc.tensor.matmul(
                        psum_d[:, s * 512:(s + 1) * 512],
                        aw_all[:, g * 128:(g + 1) * 128], rhs,
                        start=True, stop=not on_act,
                    )
                if on_act:
                    j = act_idx[pos]
                    for s in range(2):
                        if j < MACT_SPLIT:
                            mrhs = mi0_t[:, D + j * CH + s * 512:D + j * CH + s * 512 + 512]
                        else:
                            jj = j - MACT_SPLIT
                            mrhs = mi1_t[:, jj * CH + s * 512:jj * CH + s * 512 + 512]
                        nc.tensor.matmul(
                            psum_d[:, s * 512:(s + 1) * 512],
                            mi0_t[:, 0:D], mrhs,
                            start=False, stop=True,
                        )
                    scr1 = scr_pool.tile([128, CH], bf16, tag="scr")
                    scr2 = scr_pool.tile([128, CH], bf16, tag="scr")
                    nc.scalar.activation(
                        scr1[:], psum_d[:], Act.Relu,
                        bias=bias_a[:], scale=1.0,
                        accum_out=accA_t[:, 2 * j:2 * j + 1],
                    )
                    nc.scalar.activation(
                        scr2[:], psum_d[:], Act.Relu,
                        bias=bias_b[:], scale=-LAM,
                        accum_out=accA_t[:, 2 * j + 1:2 * j + 2],
                    )
                else:
                    gq, mcol = dve_loc[pos]
                    src1 = (lead_m_t[:, mcol:mcol + CH] if gq < 0
                            else mgrp[gq][:, mcol:mcol + CH])
                    scr = scr_pool.tile([128, CH], bf16, tag="scr")
                    nc.vector._custom_dve(
                        _HINGE_OP,
                        out=scr[:], in0=psum_d[:], in1=src1,
                        s0=MOFF - 5.0, s1=LAM,
                        accum_out=accD_t[:, n_dve:n_dve + 1],
                    )
                    n_dve += 1

            nc.sync.dma_start(accs_out[:, :32], accD_t[:])
            nc.scalar.dma_start(accs_out[:, 32:], accA_t[:])

    nc.finalize()
    return nc


def _prep_inputs(descriptors_0, descriptors_1, similarity_mask):
    d0 = np.asarray(descriptors_0, dtype=np.float32)
    d1 = np.asarray(descriptors_1, dtype=np.float32)
    mkv = np.asarray(similarity_mask)
    idn128 = (-MOFF * np.eye(D, dtype=np.float32)).astype(ml_dtypes.float8_e5m2)
    in_maps = []
    n_act = len(ACT_POS)
    for c in range(N_CORES):
        b = c >> 2
        isl = (c & 3) * 16
        aw128 = (
            d0[b].reshape(D, IJ)[:, isl * W:(isl + 16) * W] * np.float32(5.0)
        ).astype(ml_dtypes.float8_e4m3)
        bm128 = d1[b].reshape(D, IJ).astype(ml_dtypes.float8_e4m3)
        m = mkv[b, isl:isl + 16].reshape(ROWS_PER_CORE, IJ)
        mq = m.reshape(G, 128, KT, CH).transpose(0, 2, 1, 3).reshape(N_CHUNKS, 128, CH)
        # DVE masks: t-form {1, 8192} fp8e5m2, in processing order
        dm = [np.where(mq[ORDER[p]], np.float32(MOFF), np.float32(1.0)).astype(
            ml_dtypes.float8_e5m2) for p in DVE_POS]
        mvv = {}
        off = DVE_GROUPS[0]
        for i, n in enumerate(DVE_GROUPS[1:]):
            grp = np.stack(dm[off:off + n])  # [n, 128, CH]
            mvv[f"mv{i}"] = np.ascontiguousarray(
                grp.transpose(1, 0, 2).reshape(128, n * CH)
            )
            off += n
        # ACT masks: {0,1} fp8e5m2, in processing order
        mact = np.stack([mq[ORDER[p]].astype(ml_dtypes.float8_e5m2)
                         for p in ACT_POS])   # [n_act, 128, CH]
        mact = mact.transpose(1, 0, 2)        # [128, n_act, CH]
        im = {
            "lead_a": np.ascontiguousarray(
                np.concatenate([aw128, bm128[:, :CH]], axis=1)),
            "lead_m": np.ascontiguousarray(np.concatenate([dm[0], dm[1]], axis=1)),
            "b1": np.ascontiguousarray(bm128[:, CH:]),
            "mi0": np.ascontiguousarray(np.concatenate(
                [idn128, mact[:, :MACT_SPLIT].reshape(128, MACT_SPLIT * CH)],
                axis=1)),
            "mi1": np.ascontiguousarray(
                mact[:, MACT_SPLIT:].reshape(128, (n_act - MACT_SPLIT) * CH)),
        }
        im.update(mvv)
        in_maps.append(im)
    return in_maps


def _run(in_maps, **kwargs):
    if "nc" not in _cached:
        _cached["nc"] = _build_program()
    return run_bass_kernel_spmd(_cached["nc"], in_maps, list(range(N_CORES)), **kwargs)


def _combine(results):
    total = 0.0
    n_act = len(ACT_POS)
    for r in results:
        accs = r["accs"].astype(np.float64)
        total += accs[:, :23].sum() + accs[:, 32:32 + 2 * n_act].sum()
    return np.float32(total / 5.0 / float(B * IJ * IJ))


def kernel(descriptors_0, descriptors_1, similarity_mask):
    in_maps = _prep_inputs(descriptors_0, descriptors_1, similarity_mask)
    res = _run(in_maps)
    return _combine(res.results)


# revision 29
# speedup vs baseline: 1.0372x; 1.0372x over previous
"""DescriptorLoss kernel for Trainium2 (8 NeuronCores, SPMD data-parallel).

Math:
    d[b,ij,kl] = sum_c desc0[b,c,ij] * desc1[b,c,kl]
    loss = mean(where(mask, 250*relu(1 - d), relu(d - 0.2)))

Per core (shard = (batch, i-slab) -> 1024 ij rows x 4096 kl cols), the PE
computes d' = 5*d via fp8 matmuls into PSUM fp32 in 32 chunks of
[128 x 1024] (psum pool depth 4).  In d' units the hinges sit at 1 and 5:
    5*loss_elem = relu(d'-1)        if m == 0
                  250*relu(5-d')    if m == 1

23 chunks go to the DVE (one fused custom op per chunk, Src1 = t =
(m ? 8192 : 1) fp8 e5m2):
    body = relu(d' - t) + relu((t - d' - 8187) * 250),  accum = sum
  m=0: relu(d'-1); m=1: 250*relu(5-d').  One PSUM read per element.

9 chunks go to ACT: the PE injects the mask into PSUM
(psum += (-8192*I).T @ m) giving dM = d' - 8192*m; ACT runs two relu
passes with the 250 weight folded into the free affine:
    acc1 = sum relu(dM - 1)              = sum_{m=0} relu(d'-1)
    acc2 = sum relu(-250*dM - 250*8187)  = 250 * sum_{m=1} relu(5-d')

Scheduling notes (learned from traces):
  - Chunks are processed h-minor (all h=0 column blocks first), so the
    single leading DMA (aw ++ b[:, :1024] ++ first two DVE masks) feeds
    the first 8 matmuls and the hinge pipeline starts ~1us after data
    lands.  All transfers ride ONE sync-HWDGE ring in exact consumption
    order: packet-level round-robin between rings/queues would otherwise
    delay early transfers by later ones.
  - Exactly 8 input DMAs: the Tile scheduler has 8 DMA-completion
    semaphore lanes; more input DMAs alias lanes and create false
    multi-microsecond waits on unrelated transfers.
  - The scalar engine issues no DMAs (descriptor generation costs
    ~0.7us/DMA on the issuing engine) so ACT is free for hinge passes.
  - No PE warmup: cold matmul rate still outpaces the DVE hinge stream,
    and sustained real matmuls open the HAM clock gate on their own.
"""

import numpy as np
import ml_dtypes
from operator import add

import concourse.bacc as bacc
import concourse.mybir as mybir
import concourse.tile as tile
import concourse.dve_ops as dve_ops_mod
from concourse.dve_spec import Spec, Src0, Src1, C0, C1, relu, lower
from concourse.dve_uop import DveOpSpec
from concourse.bass_utils import run_bass_kernel_spmd

B, D, H, W = 2, 128, 64, 64
N_CORES = 8
IJ = H * W                # 4096
ROWS_PER_CORE = IJ // 4   # 1024
G = ROWS_PER_CORE // 128  # 8 row groups of 128
CH = 1024                 # chunk columns
KT = IJ // CH             # 4 chunks per row group
N_CHUNKS = G * KT         # 32
MOFF = 8192.0             # mask offset (exact in fp8 e5m2)
LAM = 250.0

# processing order: h-minor (all h=0 chunks first), g-major within h
ORDER = tuple(g * KT + h for h in range(KT) for g in range(G))
ACT_POS = (2, 5, 9, 12, 16, 19, 23, 26, 30)      # positions on ACT (9)
DVE_POS = tuple(p for p in range(N_CHUNKS) if p not in ACT_POS)
# DVE mask windows, in processing order: 2 chunks ride in the lead DMA
DVE_GROUPS = (2, 3, 4, 5, 5, 4)
MACT_SPLIT = 2                                   # ACT chunks in mi0

_cached = {}

_OP_NAME = "HINGE_PAIR_MASKED_ANT"


def _hinge_ref(in0, in1, s0, s1, imm2):
    x = in0.astype(np.float32)
    t = in1.astype(np.float32)
    out = np.maximum(x - t, 0) + np.maximum((t - x - s0) * s1, 0)
    return out, out.reshape(out.shape[0], -1).sum(axis=-1, keepdims=True).astype(
        np.float32
    )


def _register_dve_op():
    """Register the fused two-hinge op in dve_ops.OPS (documented extension
    point; the uop table is emitted per-NEFF at compile time)."""
    for op in dve_ops_mod.OPS:
        if op.name == _OP_NAME:
            return op
    spec = Spec(
        body=relu(Src0 - Src1) + relu((Src1 - Src0 - C0) * C1),
        accum=add,
        reference=_hinge_ref,
    )
    opcode = dve_ops_mod._CUSTOM_DVE_ROW_BASE + len(dve_ops_mod.OPS)
    shas = {}
    for ver in ("v3", "v4"):
        shas[ver] = DveOpSpec(
            name=_OP_NAME, opcode=opcode, uops=lower(spec, ver=ver), rd1_en=True
        ).sha(ver)
    op = dve_ops_mod.DveOp(_OP_NAME, spec, subdim=False, uops_sha=shas)
    dve_ops_mod.OPS.append(op)
    dve_ops_mod._SUB_OPCODE_FOR_NAME[_OP_NAME] = opcode
    dve_ops_mod.CUSTOM_DVE_SPECS[_OP_NAME] = spec
    return op


_HINGE_OP = _register_dve_op()


def _build_program():
    nc = bacc.Bacc("TRN2")
    f32 = mybir.dt.float32
    bf16 = mybir.dt.bfloat16
    f8 = mybir.dt.float8e5
    f8e4 = mybir.dt.float8e4
    Act = mybir.ActivationFunctionType
    n_act = len(ACT_POS)

    # lead_a = aw [128,1024] ++ b[:, :1024]; lead_m = first 2 DVE masks
    lead_a = nc.declare_dram_parameter("lead_a", [D, 2 * CH], f8e4, isOutput=False)
    lead_m = nc.declare_dram_parameter("lead_m", [D, 2 * CH], f8, isOutput=False)
    b1 = nc.declare_dram_parameter("b1", [D, 3 * CH], f8e4, isOutput=False)
    mi0 = nc.declare_dram_parameter(
        "mi0", [D, D + MACT_SPLIT * CH], f8, isOutput=False)
    mi1 = nc.declare_dram_parameter(
        "mi1", [D, (n_act - MACT_SPLIT) * CH], f8, isOutput=False)
    mvs = [
        nc.declare_dram_parameter(f"mv{i}", [128, n * CH], f8, isOutput=False)
        for i, n in enumerate(DVE_GROUPS[1:])
    ]
    accs_out = nc.declare_dram_parameter("accs", [128, 64], f32, isOutput=True)

    # DVE position -> (window, col offset); window -1 = lead DMA
    dve_loc = {}
    k = 0
    for gq, n in enumerate(DVE_GROUPS):
        for idx in range(n):
            dve_loc[DVE_POS[k]] = (gq - 1, idx * CH)
            k += 1
    act_idx = {p: j for j, p in enumerate(ACT_POS)}

    with tile.TileContext(nc) as tc:
        with (
            tc.tile_pool(name="desc", bufs=1) as desc_pool,
            tc.tile_pool(name="mask", bufs=6) as mask_pool,
            tc.tile_pool(name="scr", bufs=4) as scr_pool,
            tc.tile_pool(name="accs", bufs=1) as acc_pool,
            tc.tile_pool(name="psd", bufs=4, space="PSUM") as psum_pool,
        ):
            lead_a_t = desc_pool.tile([D, 2 * CH], f8e4, tag="leada")
            lead_m_t = desc_pool.tile([D, 2 * CH], f8, tag="leadm")
            b1_t = desc_pool.tile([D, 3 * CH], f8e4, tag="b1")
            mi0_t = desc_pool.tile([D, D + MACT_SPLIT * CH], f8, tag="mi0")
            mi1_t = desc_pool.tile([D, (n_act - MACT_SPLIT) * CH], f8, tag="mi1")
            mgrp = [
                mask_pool.tile([128, n * CH], f8, tag="m", name=f"mg{i}")
                for i, n in enumerate(DVE_GROUPS[1:])
            ]
            warm = desc_pool.tile([128, 8], bf16, tag="warm")
            warm2 = desc_pool.tile([128, 8], bf16, tag="warm2")
            bias_a = desc_pool.tile([128, 1], f32, tag="ba")
            bias_b = desc_pool.tile([128, 1], f32, tag="bb")
            junk = desc_pool.tile([128, 512], f8e4, tag="junk")
            accD_t = acc_pool.tile([128, 32], f32, tag="accD")
            accA_t = acc_pool.tile([128, 32], f32, tag="accA")

            # single sync ring, exact consumption order
            nc.sync.dma_start(lead_a_t[:], lead_a[:])
            nc.sync.dma_start(lead_m_t[:], lead_m[:])
            nc.sync.dma_start(mi0_t[:], mi0[:])
            nc.sync.dma_start(mgrp[0][:], mvs[0][:])
            nc.sync.dma_start(mgrp[1][:], mvs[1][:])
            nc.sync.dma_start(b1_t[:], b1[:])
            nc.sync.dma_start(mi1_t[:], mi1[:])
            nc.sync.dma_start(mgrp[2][:], mvs[2][:])
            nc.sync.dma_start(mgrp[3][:], mvs[3][:])
            nc.sync.dma_start(mgrp[4][:], mvs[4][:])

            nc.vector.memset(junk[:], 0.0)
            nc.vector.memset(warm[:], 0.0)
            nc.vector.memset(bias_a[:], -1.0)
            nc.vector.memset(bias_b[:], -(LAM * (MOFF - 5.0)))
            nc.vector.memset(accD_t[:, 23:], 0.0)
            nc.vector.memset(accA_t[:, 2 * n_act:], 0.0)
            # prime the ACT relu table (~2.7us one-time) under the input DMAs
            nc.scalar.activation(warm2[:], warm[:], Act.Relu, bias=bias_a[:], scale=1.0)

            # HAM warmup: dense junk matmuls under the input-DMA shadow so
            # the PE clock gate is open (K=8/8) when the real matmuls begin.
            # They use the first psum pool buffer; the pos-3 chunk reuses it
            # only after they retire (~well before its data arrives).
            junk_ps = psum_pool.tile([128, CH], f32, tag="d")
            with tc.high_priority():
                for w in range(8):
                    nc.tensor.matmul(
                        junk_ps[:, (w % 2) * 512:(w % 2) * 512 + 512],
                        junk[:, 0:128], junk[:],
                        start=True, stop=True,
                    )

            aw_all = lead_a_t[:, 0:CH]
            b0 = lead_a_t[:, CH:2 * CH]

            n_dve = 0
            for pos in range(N_CHUNKS):
                cid = ORDER[pos]
                on_act = pos in ACT_POS
                g, h = divmod(cid, KT)
                psum_d = psum_pool.tile([128, CH], f32, tag="d")
                for s in range(2):
                    if h == 0:
                        rhs = b0[:, s * 512:(s + 1) * 512]
                    else:
                        c0 = (h - 1) * CH + s * 512
                        rhs = b1_t[:, c0:c0 + 512]
                    nc.tensor.matmul(
                        psum_d[:, s * 512:(s + 1) * 512],
                        aw_all[:, g * 128:(g + 1) * 128], rhs,
                        start=True, stop=not on_act,
                    )
                if on_act:
                    j = act_idx[pos]
                    for s in range(2):
                        if j < MACT_SPLIT:
                            mrhs = mi0_t[:, D + j * CH + s * 512:D + j * CH + s * 512 + 512]
                        else:
                            jj = j - MACT_SPLIT
                            mrhs = mi1_t[:, jj * CH + s * 512:jj * CH + s * 512 + 512]
                        nc.tensor.matmul(
                            psum_d[:, s * 512:(s + 1) * 512],
                            mi0_t[:, 0:D], mrhs,
                            start=False, stop=True,
                        )
                    scr1 = scr_pool.tile([128, CH], bf16, tag="scr")
                    scr2 = scr_pool.tile([128, CH], bf16, tag="scr")
                    nc.scalar.activation(
                        scr1[:], psum_d[:], Act.Relu,
                        bias=bias_a[:], scale=1.0,
                        accum_out=accA_t[:, 2 * j:2 * j + 1],
                    )
                    nc.scalar.activation(
                        scr2[:], psum_d[:], Act.Relu,
                        bias=bias_b[:], scale=-LAM,
                        accum_out=accA_t[:, 2 * j + 1:2 * j + 2],
                    )
                else:
                    gq, mcol = dve_loc[pos]
                    src1 = (lead_m_t[:, mcol:mcol + CH] if gq < 0
                            else mgrp[gq][:, mcol:mcol + CH])
                    scr = scr_pool.tile([128, CH], bf16, tag="scr")
                    nc.vector._custom_dve(
                        _HINGE_OP,
                        out=scr[:], in0=psum_d[:], in1=src1,
                        s0=MOFF - 5.0, s1=LAM,
                        accum_out=accD_t[:, n_dve:n_dve + 1],
                    )
                    n_dve += 1

            nc.sync.dma_start(accs_out[:, :32], accD_t[:])
            nc.scalar.dma_start(accs_out[:, 32:], accA_t[:])

    nc.finalize()
    return nc


def _prep_inputs(descriptors_0, descriptors_1, similarity_mask):
    d0 = np.asarray(descriptors_0, dtype=np.float32)
    d1 = np.asarray(descriptors_1, dtype=np.float32)
    mkv = np.asarray(similarity_mask)
    idn128 = (-MOFF * np.eye(D, dtype=np.float32)).astype(ml_dtypes.float8_e5m2)
    in_maps = []
    n_act = len(ACT_POS)
    for c in range(N_CORES):
        b = c >> 2
        isl = (c & 3) * 16
        aw128 = (
            d0[b].reshape(D, IJ)[:, isl * W:(isl + 16) * W] * np.float32(5.0)
        ).astype(ml_dtypes.float8_e4m3)
        bm128 = d1[b].reshape(D, IJ).astype(ml_dtypes.float8_e4m3)
        m = mkv[b, isl:isl + 16].reshape(ROWS_PER_CORE, IJ)
        mq = m.reshape(G, 128, KT, CH).transpose(0, 2, 1, 3).reshape(N_CHUNKS, 128, CH)
        # DVE masks: t-form {1, 8192} fp8e5m2, in processing order
        dm = [np.where(mq[ORDER[p]], np.float32(MOFF), np.float32(1.0)).astype(
            ml_dtypes.float8_e5m2) for p in DVE_POS]
        mvv = {}
        off = DVE_GROUPS[0]
        for i, n in enumerate(DVE_GROUPS[1:]):
            grp = np.stack(dm[off:off + n])  # [n, 128, CH]
            mvv[f"mv{i}"] = np.ascontiguousarray(
                grp.transpose(1, 0, 2).reshape(128, n * CH)
            )
            off += n
        # ACT masks: {0,1} fp8e5m2, in processing order
        mact = np.stack([mq[ORDER[p]].astype(ml_dtypes.float8_e5m2)
                         for p in ACT_POS])   # [n_act, 128, CH]
        mact = mact.transpose(1, 0, 2)        # [128, n_act, CH]
        im = {
            "lead_a": np.ascontiguousarray(
                np.concatenate([aw128, bm128[:, :CH]], axis=1)),
            "lead_m": np.ascontiguousarray(np.concatenate([dm[0], dm[1]], axis=1)),
            "b1": np.ascontiguousarray(bm128[:, CH:]),
            "mi0": np.ascontiguousarray(np.concatenate(
                [idn128, mact[:, :MACT_SPLIT].reshape(128, MACT_SPLIT * CH)],
                axis=1)),
            "mi1": np.ascontiguousarray(
                mact[:, MACT_SPLIT:].reshape(128, (n_act - MACT_SPLIT) * CH)),
        }
        im.update(mvv)
        in_maps.append(im)
    return in_maps


def _run(in_maps, **kwargs):
    if "nc" not in _cached:
        _cached["nc"] = _build_program()
    return run_bass_kernel_spmd(_cached["nc"], in_maps, list(range(N_CORES)), **kwargs)


def _combine(results):
    total = 0.0
    n_act = len(ACT_POS)
    for r in results:
        accs = r["accs"].astype(np.float64)
        total += accs[:, :23].sum() + accs[:, 32:32 + 2 * n_act].sum()
    return np.float32(total / 5.0 / float(B * IJ * IJ))


def kernel(descriptors_0, descriptors_1, similarity_mask):
    in_maps = _prep_inputs(descriptors_0, descriptors_1, similarity_mask)
    res = _run(in_maps)
    return _combine(res.results)
